# revision 19
# baseline (speedup 1.0000x reference)
"""Multi-scale voxel feature lookup + per-level projector MLP on 8 TRN2 cores.

Strategy: channel-shard the 5 feature volumes 8 ways (16 channels/core).
Host pre-transposes each level's shard to channel-last rows and concatenates
all levels into one (1198336, 16) row table, so the whole lookup is a single
20-row indirect DMA driven by device-computed indices. Each core computes the
flat row indices from p (int32 vector ops on a host-packed (20,10) tensor of
p + shift/weight/base constants), gathers its 16-channel shard of all 20
(level, sample) vectors, AllGathers the 1.25KB shards across the chip, then
every core runs the full 5-level MLP (conv1x1 -> train-mode BN -> ReLU ->
conv1x1) redundantly and writes an identical (5,4,64) output; the host
returns core 0's copy. All weights arrive as one host-packed SBUF-layout
tensor (one DMA).

Note: b1 cancels exactly in train-mode BN (h - mean(h)), so it is unused.
"""

import numpy as np

import concourse.bass as bass
import concourse.tile as tile
from concourse import bacc, mybir
from concourse.bass_utils import run_bass_kernel_spmd

N_CORES = 8
BS = 4
C = 128
CS = C // N_CORES  # 16 channels per core
HID = 256
OUT = 64
SIDES = [64, 32, 16, 8, 4]
NLVL = 5
EPS = 1e-5
NROW = NLVL * BS  # 20 gathered rows
NG = 2 * NLVL  # (level, half) groups: hidden 256 split into 2 partition halves

# row offsets of each level's block in the concatenated table
BASE = np.concatenate([[0], np.cumsum([BS * s**3 for s in SIDES])]).astype(np.int64)
TOTAL_ROWS = int(BASE[-1])

# wpack column blocks
W1_OFF = 0
W2_OFF = W1_OFF + NLVL * HID          # 1280
G_OFF = W2_OFF + NG * OUT             # 1920
B_OFF = G_OFF + NG                    # 1930
B2_OFF = B_OFF + NG                   # 1940
WCOLS = B2_OFF + NLVL * OUT           # 2260

F32 = mybir.dt.float32
I32 = mybir.dt.int32


def _pk_consts() -> np.ndarray:
    """(20, 10) int32 rows r = l*4+b: [p placeholder (3) | shift (3) | wmul (3) | base]."""
    pk = np.zeros((NROW, 10), dtype=np.int32)
    for l, s in enumerate(SIDES):
        for b in range(BS):
            r = l * BS + b
            pk[r, 3:6] = l + 1
            pk[r, 6:9] = [s * s, s, 1]
            pk[r, 9] = BASE[l] + b * s**3
    return pk


_PK_CONST = _pk_consts()


def build_program(collective=True, big_tables=True, gathers=True, n_dev=N_CORES):
    nc = bacc.Bacc("TRN2", target_bir_lowering=False, debug=False, num_devices=n_dev)

    nrows = TOTAL_ROWS if big_tables else 64
    xall = nc.dram_tensor("xall", [nrows, CS], F32, kind="ExternalInput").ap()
    pk = nc.dram_tensor("pk", [NROW, 10], I32, kind="ExternalInput").ap()
    wpack = nc.dram_tensor("wpack", [C, WCOLS], F32, kind="ExternalInput").ap()
    out = nc.dram_tensor("out", [NLVL, BS, OUT], F32, kind="ExternalOutput").ap()

    # j-major collective buffers: cc row j, col r=(l*4+b); cc_out row (c*16+j)
    cc_in = nc.dram_tensor("cc_in", [CS, NROW], F32).ap()
    cc_out = nc.dram_tensor("cc_out", [C, NROW], F32, addr_space="Shared").ap()

    with tile.TileContext(nc) as tc:
        with (
            tc.tile_pool(name="sbuf", bufs=1) as sp,
            tc.tile_pool(name="psum", bufs=1, space="PSUM") as pp,
        ):
            # ---- two input DMAs on separate queues
            pkt = sp.tile([NROW, 10], I32, tag="pkt")
            nc.sync.dma_start(pkt[:], pk)
            wsb = sp.tile([C, WCOLS], F32, tag="wsb")
            nc.scalar.dma_start(wsb[:], wpack)
            w1v = wsb[:, W1_OFF : W1_OFF + NLVL * HID]
            w2v = wsb[:, W2_OFF : W2_OFF + NG * OUT]
            gsb = wsb[:, G_OFF : G_OFF + NG]
            bsb = wsb[:, B_OFF : B_OFF + NG]
            b2v = wsb[0:BS, B2_OFF : B2_OFF + NLVL * OUT]

            # ---- index math: idx[r] = sum_t (p[r,t] >> sh[r]) * w[r,t] + base[r]
            q20 = sp.tile([NROW, 3], I32, tag="q20")
            nc.vector.tensor_tensor(
                out=q20[:], in0=pkt[:, 0:3], in1=pkt[:, 3:6],
                op=mybir.AluOpType.logical_shift_right,
            )
            m20 = sp.tile([NROW, 3], I32, tag="m20")
            nc.vector.tensor_tensor(
                out=m20[:], in0=q20[:], in1=pkt[:, 6:9], op=mybir.AluOpType.mult
            )
            s20 = sp.tile([NROW, 1], I32, tag="s20")
            with nc.allow_low_precision(reason="exact int32 index sums"):
                nc.vector.reduce_sum(out=s20[:], in_=m20[:], axis=mybir.AxisListType.X)
            idx20 = sp.tile([NROW, 1], I32, tag="idx20")
            nc.vector.tensor_tensor(
                out=idx20[:], in0=s20[:], in1=pkt[:, 9:10], op=mybir.AluOpType.add
            )

            # ---- one indirect row gather: feats20[r, :] = xall[idx20[r], :]
            feats20 = sp.tile([NROW, CS], F32, tag="feats20")
            if gathers:
                nc.gpsimd.indirect_dma_start(
                    out=feats20[:],
                    out_offset=None,
                    in_=xall,
                    in_offset=bass.IndirectOffsetOnAxis(ap=idx20[:, 0:1], axis=0),
                )
            else:
                nc.sync.dma_start(feats20[:], xall[0:NROW, :])

            # ---- AllGather the per-core channel shards
            # stage into a 32x32 tile so one DVE block-transpose yields the
            # j-major layout for a contiguous collective write
            f32sq = sp.tile([32, 32], F32, tag="f32sq")
            nc.gpsimd.memset(f32sq[:], 0.0)
            nc.vector.tensor_copy(f32sq[0:NROW, 0:CS], feats20[:])
            f32sqT = sp.tile([32, 32], F32, tag="f32sqT")
            nc.vector.transpose(f32sqT[:], f32sq[:])
            nc.sync.dma_start(cc_in, f32sqT[0:CS, 0:NROW])
            fT = sp.tile([C, NROW], F32, tag="fT")
            if collective:
                nc.gpsimd.collective_compute(
                    "AllGather",
                    mybir.AluOpType.bypass,
                    replica_groups=[list(range(N_CORES))],
                    ins=[cc_in],
                    outs=[cc_out],
                )
                nc.sync.dma_start(fT[:], cc_out)
            else:
                for c in range(N_CORES):
                    nc.sync.dma_start(fT[c * CS : (c + 1) * CS, :], cc_in)

            # ---- matmul 1: psum1[:, g*4:+4] = w1 half (l,h).T @ fT level l
            psum1 = pp.tile([C, NG * BS], F32, tag="psum1")
            for h in range(2):
                for l in range(NLVL):
                    g = l * 2 + h
                    nc.tensor.matmul(
                        out=psum1[:, g * BS : (g + 1) * BS],
                        lhsT=w1v[:, l * HID + h * C : l * HID + h * C + C],
                        rhs=fT[:, l * BS : (l + 1) * BS],
                        start=True,
                        stop=True,
                    )

            # ---- train-mode batch norm over the batch (free) axis + ReLU
            # r = relu(h*gsc + (beta - mean*gsc)), gsc = gamma/sqrt(var+eps)
            # var+eps = (E[h^2]+eps) - mean^2, both moments in two passes over h
            p1v = psum1[:].rearrange("p (g b) -> p g b", g=NG)
            sums = sp.tile([C, NG], F32, tag="sums")
            nc.vector.reduce_sum(
                out=sums[:].rearrange("p (g o) -> p g o", o=1),
                in_=p1v,
                axis=mybir.AxisListType.X,
            )
            scr = sp.tile([C, NG * BS], F32, tag="scr")
            nc.scalar.activation(scr[:], psum1[:], mybir.ActivationFunctionType.Square)
            vs = sp.tile([C, NG], F32, tag="vs")
            nc.vector.reduce_sum(
                out=vs[:].rearrange("p (g o) -> p g o", o=1),
                in_=scr[:].rearrange("p (g b) -> p g b", g=NG),
                axis=mybir.AxisListType.X,
            )
            vpe = sp.tile([C, NG], F32, tag="vpe")
            nc.vector.tensor_scalar(
                out=vpe[:],
                in0=vs[:],
                scalar1=1.0 / BS,
                scalar2=EPS,
                op0=mybir.AluOpType.mult,
                op1=mybir.AluOpType.add,
            )
            mean = sp.tile([C, NG], F32, tag="mean")
            nc.vector.tensor_scalar_mul(mean[:], sums[:], 1.0 / BS)
            ms = sp.tile([C, NG], F32, tag="ms")
            nc.vector.tensor_mul(ms[:], mean[:], mean[:])
            nc.vector.tensor_sub(vpe[:], vpe[:], ms[:])
            std = sp.tile([C, NG], F32, tag="std")
            nc.scalar.activation(std[:], vpe[:], mybir.ActivationFunctionType.Sqrt)
            inv = sp.tile([C, NG], F32, tag="inv")
            nc.vector.reciprocal(inv[:], std[:])
            gsc = sp.tile([C, NG], F32, tag="gsc")
            nc.vector.tensor_mul(gsc[:], inv[:], gsb)
            mg = sp.tile([C, NG], F32, tag="mg")
            nc.vector.tensor_mul(mg[:], mean[:], gsc[:])
            b2g = sp.tile([C, NG], F32, tag="b2g")
            nc.vector.tensor_sub(b2g[:], bsb, mg[:])

            r = sp.tile([C, NG * BS], F32, tag="r")
            rv = r[:].rearrange("p (g b) -> p g b", g=NG)
            gscb = gsc[:].rearrange("p (g o) -> p g o", o=1).to_broadcast([C, NG, BS])
            nc.vector.tensor_tensor(out=rv, in0=p1v, in1=gscb, op=mybir.AluOpType.mult)
            b2gb = b2g[:].rearrange("p (g o) -> p g o", o=1).to_broadcast([C, NG, BS])
            nc.vector.tensor_tensor(out=rv, in0=rv, in1=b2gb, op=mybir.AluOpType.add)
            nc.vector.tensor_scalar_max(r[:], r[:], 0.0)

            # ---- matmul 2: out_l = r_l.T @ w2_l (accumulate the two hidden halves)
            psum2 = pp.tile([BS, NLVL * OUT], F32, tag="psum2")
            for l in range(NLVL):
                for h in range(2):
                    g = l * 2 + h
                    nc.tensor.matmul(
                        out=psum2[:, l * OUT : (l + 1) * OUT],
                        lhsT=r[:, g * BS : (g + 1) * BS],
                        rhs=w2v[:, g * OUT : (g + 1) * OUT],
                        start=(h == 0),
                        stop=(h == 1),
                    )

            osb = sp.tile([BS, NLVL * OUT], F32, tag="osb")
            nc.vector.tensor_add(osb[:], psum2[:], b2v)
            nc.scalar.dma_start(
                out.rearrange("l b o -> b l o"),
                osb[:].rearrange("b (l o) -> b l o", l=NLVL),
            )

    nc.compile()
    return nc


def shard_inputs(x0, x1, x2, x3, x4, p, w1, gamma, beta, w2, b2):
    """Build the 8 per-core input maps (numpy only, no index-dependent slicing)."""
    xs = [x0, x1, x2, x3, x4]
    w1_np = np.asarray(w1, dtype=np.float32)
    w2_np = np.asarray(w2, dtype=np.float32)

    wpack = np.zeros((C, WCOLS), dtype=np.float32)
    # w1sb[k, l*HID + m] = w1[l, m, k]
    wpack[:, W1_OFF : W1_OFF + NLVL * HID] = (
        w1_np.transpose(2, 0, 1).reshape(C, NLVL * HID)
    )
    # w2sb[j, (l*2+h)*OUT + o] = w2[l, o, h*C+j]
    w2r = w2_np.transpose(2, 0, 1).reshape(2, C, NLVL, OUT)  # (h, j, l, o)
    wpack[:, W2_OFF : W2_OFF + NG * OUT] = (
        w2r.transpose(1, 2, 0, 3).reshape(C, NG * OUT)
    )
    gb = np.asarray(gamma, dtype=np.float32).reshape(NLVL, 2, C)  # (l, h, j)
    wpack[:, G_OFF : G_OFF + NG] = gb.transpose(2, 0, 1).reshape(C, NG)
    bb = np.asarray(beta, dtype=np.float32).reshape(NLVL, 2, C)
    wpack[:, B_OFF : B_OFF + NG] = bb.transpose(2, 0, 1).reshape(C, NG)
    wpack[0:BS, B2_OFF : B2_OFF + NLVL * OUT] = np.broadcast_to(
        np.asarray(b2, dtype=np.float32).reshape(1, NLVL * OUT), (BS, NLVL * OUT)
    )

    pk = _PK_CONST.copy()
    pk[:, 0:3] = np.tile(np.asarray(p).astype(np.int32), (NLVL, 1))

    in_maps = []
    for c in range(N_CORES):
        xall = np.empty((TOTAL_ROWS, CS), dtype=np.float32)
        for l, x in enumerate(xs):
            shard = np.asarray(x[:, c * CS : (c + 1) * CS], dtype=np.float32)
            xall[BASE[l] : BASE[l + 1]] = shard.transpose(0, 2, 3, 4, 1).reshape(
                BASE[l + 1] - BASE[l], CS
            )
        in_maps.append({"xall": xall, "pk": pk, "wpack": wpack})
    return in_maps


_NC_CACHE = None


def kernel(x0, x1, x2, x3, x4, p, w1, b1, gamma, beta, w2, b2):
    global _NC_CACHE
    if _NC_CACHE is None:
        _NC_CACHE = build_program()
    nc = _NC_CACHE
    in_maps = shard_inputs(x0, x1, x2, x3, x4, p, w1, gamma, beta, w2, b2)
    res = run_bass_kernel_spmd(nc, in_maps, list(range(N_CORES)))
    return np.asarray(res.results[0]["out"], dtype=np.float32)


# revision 26
# speedup vs baseline: 1.6769x; 1.6769x over previous
"""Multi-scale voxel feature lookup + per-level projector MLP on 8 TRN2 cores.

Strategy: channel-shard the 5 feature volumes 8 ways (16 channels/core).
Host pre-transposes each level's shard to channel-last rows and concatenates
all levels into one (1198336, 16) row table, so the whole lookup is a single
20-row indirect DMA driven by device-computed indices. Each core computes the
flat row indices from p (int32 vector ops on a host-packed (20,10) tensor of
p + shift/weight/base constants), gathers its 16-channel shard of all 20
(level, sample) vectors, AllGathers the 1.25KB shards across the chip, then
every core runs the full 5-level MLP (conv1x1 -> train-mode BN -> ReLU ->
conv1x1) redundantly and writes an identical (5,4,64) output; the host
returns core 0's copy. All weights arrive as one host-packed SBUF-layout
tensor (one DMA).

Note: b1 cancels exactly in train-mode BN (h - mean(h)), so it is unused.
"""

import numpy as np

import concourse.bass as bass
import concourse.tile as tile
from concourse import bacc, mybir
from concourse.bass_utils import run_bass_kernel_spmd

N_CORES = 8
BS = 4
C = 128
CS = C // N_CORES  # 16 channels per core
HID = 256
OUT = 64
SIDES = [64, 32, 16, 8, 4]
NLVL = 5
EPS = 1e-5
NROW = NLVL * BS  # 20 gathered rows
NG = 2 * NLVL  # (level, half) groups: hidden 256 split into 2 partition halves

# row offsets of each level's block in the concatenated table
BASE = np.concatenate([[0], np.cumsum([BS * s**3 for s in SIDES])]).astype(np.int64)
TOTAL_ROWS = int(BASE[-1])

# wpack column blocks
W1_OFF = 0
W2_OFF = W1_OFF + NLVL * HID          # 1280
G_OFF = W2_OFF + NG * OUT             # 1920
B_OFF = G_OFF + NG                    # 1930
B2_OFF = B_OFF + NG                   # 1940
WCOLS = B2_OFF + NROW                 # 1960  (b2 stored transposed: [o, l*4+b])

F32 = mybir.dt.float32
I32 = mybir.dt.int32


def _pk_consts() -> np.ndarray:
    """(20, 10) int32 rows r = l*4+b: [p placeholder (3) | shift (3) | wmul (3) | base]."""
    pk = np.zeros((NROW, 10), dtype=np.int32)
    for l, s in enumerate(SIDES):
        for b in range(BS):
            r = l * BS + b
            pk[r, 3:6] = l + 1
            pk[r, 6:9] = [s * s, s, 1]
            pk[r, 9] = BASE[l] + b * s**3
    return pk


_PK_CONST = _pk_consts()


def build_program(collective=True, big_tables=True, gathers=True, n_dev=N_CORES):
    nc = bacc.Bacc("TRN2", target_bir_lowering=False, debug=False, num_devices=n_dev)

    nrows = TOTAL_ROWS if big_tables else 64
    xall = nc.dram_tensor("xall", [nrows, CS], F32, kind="ExternalInput").ap()
    pk = nc.dram_tensor("pk", [NROW, 10], I32, kind="ExternalInput").ap()
    wpack = nc.dram_tensor("wpack", [C, WCOLS], F32, kind="ExternalInput").ap()
    out = nc.dram_tensor("out", [NLVL, BS, OUT], F32, kind="ExternalOutput").ap()

    # j-major collective buffers: cc row j, col r=(l*4+b); cc_out row (c*16+j)
    cc_in = nc.dram_tensor("cc_in", [CS, NROW], F32).ap()
    cc_out = nc.dram_tensor("cc_out", [C, NROW], F32, addr_space="Shared").ap()

    with tile.TileContext(nc) as tc:
        with (
            tc.tile_pool(name="sbuf", bufs=1) as sp,
            tc.tile_pool(name="psum", bufs=1, space="PSUM") as pp,
        ):
            # ---- two input DMAs on separate queues
            pkt = sp.tile([NROW, 10], I32, tag="pkt")
            nc.sync.dma_start(pkt[:], pk)
            wsb = sp.tile([C, WCOLS], F32, tag="wsb")
            nc.scalar.dma_start(wsb[:], wpack)
            w1v = wsb[:, W1_OFF : W1_OFF + NLVL * HID]
            w2v = wsb[:, W2_OFF : W2_OFF + NG * OUT]
            gsb = wsb[:, G_OFF : G_OFF + NG]
            bsb = wsb[:, B_OFF : B_OFF + NG]
            b2v = wsb[0:OUT, B2_OFF : B2_OFF + NROW]

            # ---- index math: idx[r] = sum_t (p[r,t] >> sh[r]) * w[r,t] + base[r]
            q20 = sp.tile([NROW, 3], I32, tag="q20")
            nc.vector.tensor_tensor(
                out=q20[:], in0=pkt[:, 0:3], in1=pkt[:, 3:6],
                op=mybir.AluOpType.logical_shift_right,
            )
            m20 = sp.tile([NROW, 3], I32, tag="m20")
            nc.vector.tensor_tensor(
                out=m20[:], in0=q20[:], in1=pkt[:, 6:9], op=mybir.AluOpType.mult
            )
            s20 = sp.tile([NROW, 1], I32, tag="s20")
            with nc.allow_low_precision(reason="exact int32 index sums"):
                nc.vector.reduce_sum(out=s20[:], in_=m20[:], axis=mybir.AxisListType.X)
            idx20 = sp.tile([NROW, 1], I32, tag="idx20")
            nc.vector.tensor_tensor(
                out=idx20[:], in0=s20[:], in1=pkt[:, 9:10], op=mybir.AluOpType.add
            )

            # ---- one indirect row gather: feats20[r, :] = xall[idx20[r], :]
            feats20 = sp.tile([NROW, CS], F32, tag="feats20")
            if gathers:
                nc.gpsimd.indirect_dma_start(
                    out=feats20[:],
                    out_offset=None,
                    in_=xall,
                    in_offset=bass.IndirectOffsetOnAxis(ap=idx20[:, 0:1], axis=0),
                )
            else:
                nc.sync.dma_start(feats20[:], xall[0:NROW, :])

            # ---- AllGather the per-core channel shards
            # stage into a 32x32 tile so one DVE block-transpose yields the
            # j-major layout for a contiguous collective write
            f32sq = sp.tile([32, 32], F32, tag="f32sq")
            nc.gpsimd.memset(f32sq[:], 0.0)
            nc.vector.tensor_copy(f32sq[0:NROW, 0:CS], feats20[:])
            f32sqT = sp.tile([32, 32], F32, tag="f32sqT")
            nc.vector.transpose(f32sqT[:], f32sq[:])
            nc.sync.dma_start(cc_in, f32sqT[0:CS, 0:NROW])
            fT = sp.tile([C, NROW], F32, tag="fT")
            if collective:
                nc.gpsimd.collective_compute(
                    "AllGather",
                    mybir.AluOpType.bypass,
                    replica_groups=[list(range(N_CORES))],
                    ins=[cc_in],
                    outs=[cc_out],
                )
                nc.sync.dma_start(fT[:], cc_out)
            else:
                # proxy for the collective path: one same-sized fT load
                nc.sync.dma_start(
                    fT[:].rearrange("(c j) r -> c j r", c=N_CORES),
                    cc_in.rearrange("j r -> j r")[None, :, :].to_broadcast(
                        [N_CORES, CS, NROW]
                    ),
                )

            # ---- matmul 1: psum1[:, g*4:+4] = w1 half (l,h).T @ fT level l
            psum1 = pp.tile([C, NG * BS], F32, tag="psum1")
            for h in range(2):
                for l in range(NLVL):
                    g = l * 2 + h
                    nc.tensor.matmul(
                        out=psum1[:, g * BS : (g + 1) * BS],
                        lhsT=w1v[:, l * HID + h * C : l * HID + h * C + C],
                        rhs=fT[:, l * BS : (l + 1) * BS],
                        start=True,
                        stop=True,
                    )

            # ---- train-mode batch norm over the batch (free) axis + ReLU
            # r = relu(h*gsc + (beta - mean*gsc)), gsc = gamma/sqrt(var+eps)
            # var+eps = (E[h^2]+eps) - mean^2, both moments in two passes over h
            p1v = psum1[:].rearrange("p (g b) -> p g b", g=NG)
            sums = sp.tile([C, NG], F32, tag="sums")
            nc.vector.reduce_sum(
                out=sums[:].rearrange("p (g o) -> p g o", o=1),
                in_=p1v,
                axis=mybir.AxisListType.X,
            )
            # square on DVE (one-PSUM-input rule: copy h to SBUF first); this
            # keeps ACT's only table (Sqrt) loadable before the chain starts
            hsb = sp.tile([C, NG * BS], F32, tag="hsb")
            nc.vector.tensor_copy(hsb[:], psum1[:])
            scr = sp.tile([C, NG * BS], F32, tag="scr")
            nc.vector.tensor_mul(scr[:], hsb[:], hsb[:])
            vs = sp.tile([C, NG], F32, tag="vs")
            nc.vector.reduce_sum(
                out=vs[:].rearrange("p (g o) -> p g o", o=1),
                in_=scr[:].rearrange("p (g b) -> p g b", g=NG),
                axis=mybir.AxisListType.X,
            )
            vpe = sp.tile([C, NG], F32, tag="vpe")
            nc.vector.tensor_scalar(
                out=vpe[:],
                in0=vs[:],
                scalar1=1.0 / BS,
                scalar2=EPS,
                op0=mybir.AluOpType.mult,
                op1=mybir.AluOpType.add,
            )
            # ms = (sums/BS)^2 = (sums/BS^2) * sums in one fused DVE op
            ms = sp.tile([C, NG], F32, tag="ms")
            nc.vector.scalar_tensor_tensor(
                out=ms[:],
                in0=sums[:],
                scalar=1.0 / (BS * BS),
                in1=sums[:],
                op0=mybir.AluOpType.mult,
                op1=mybir.AluOpType.mult,
            )
            nc.vector.tensor_sub(vpe[:], vpe[:], ms[:])
            std = sp.tile([C, NG], F32, tag="std")
            nc.scalar.activation(std[:], vpe[:], mybir.ActivationFunctionType.Sqrt)
            inv = sp.tile([C, NG], F32, tag="inv")
            nc.vector.reciprocal(inv[:], std[:])
            gsc = sp.tile([C, NG], F32, tag="gsc")
            nc.vector.tensor_mul(gsc[:], inv[:], gsb)
            # mg = (sums/BS) * gsc fused
            mg = sp.tile([C, NG], F32, tag="mg")
            nc.vector.scalar_tensor_tensor(
                out=mg[:],
                in0=sums[:],
                scalar=1.0 / BS,
                in1=gsc[:],
                op0=mybir.AluOpType.mult,
                op1=mybir.AluOpType.mult,
            )
            b2g = sp.tile([C, NG], F32, tag="b2g")
            nc.vector.tensor_sub(b2g[:], bsb, mg[:])

            r = sp.tile([C, NG * BS], F32, tag="r")
            rv = r[:].rearrange("p (g b) -> p g b", g=NG)
            gscb = gsc[:].rearrange("p (g o) -> p g o", o=1).to_broadcast([C, NG, BS])
            nc.vector.tensor_tensor(out=rv, in0=p1v, in1=gscb, op=mybir.AluOpType.mult)
            b2gb = b2g[:].rearrange("p (g o) -> p g o", o=1).to_broadcast([C, NG, BS])
            nc.vector.tensor_tensor(out=rv, in0=rv, in1=b2gb, op=mybir.AluOpType.add)
            nc.vector.tensor_scalar_max(r[:], r[:], 0.0)

            # ---- matmul 2 (transposed): outT_l = w2_l.T @ r_l, accumulating halves
            psum2 = pp.tile([OUT, NROW], F32, tag="psum2")
            for l in range(NLVL):
                for h in range(2):
                    g = l * 2 + h
                    nc.tensor.matmul(
                        out=psum2[:, l * BS : (l + 1) * BS],
                        lhsT=w2v[:, g * OUT : (g + 1) * OUT],
                        rhs=r[:, g * BS : (g + 1) * BS],
                        start=(h == 0),
                        stop=(h == 1),
                    )

            osb = sp.tile([OUT, NROW], F32, tag="osb")
            nc.vector.tensor_add(osb[:], psum2[:], b2v)
            nc.sync.dma_start(
                out.rearrange("l b o -> o l b"),
                osb[:].rearrange("o (l b) -> o l b", l=NLVL),
            )

    nc.compile()
    return nc


def shard_inputs(x0, x1, x2, x3, x4, p, w1, gamma, beta, w2, b2):
    """Build the 8 per-core input maps (numpy only, no index-dependent slicing)."""
    xs = [x0, x1, x2, x3, x4]
    w1_np = np.asarray(w1, dtype=np.float32)
    w2_np = np.asarray(w2, dtype=np.float32)

    wpack = np.zeros((C, WCOLS), dtype=np.float32)
    # w1sb[k, l*HID + m] = w1[l, m, k]
    wpack[:, W1_OFF : W1_OFF + NLVL * HID] = (
        w1_np.transpose(2, 0, 1).reshape(C, NLVL * HID)
    )
    # w2sb[j, (l*2+h)*OUT + o] = w2[l, o, h*C+j]
    w2r = w2_np.transpose(2, 0, 1).reshape(2, C, NLVL, OUT)  # (h, j, l, o)
    wpack[:, W2_OFF : W2_OFF + NG * OUT] = (
        w2r.transpose(1, 2, 0, 3).reshape(C, NG * OUT)
    )
    gb = np.asarray(gamma, dtype=np.float32).reshape(NLVL, 2, C)  # (l, h, j)
    wpack[:, G_OFF : G_OFF + NG] = gb.transpose(2, 0, 1).reshape(C, NG)
    bb = np.asarray(beta, dtype=np.float32).reshape(NLVL, 2, C)
    wpack[:, B_OFF : B_OFF + NG] = bb.transpose(2, 0, 1).reshape(C, NG)
    wpack[0:OUT, B2_OFF : B2_OFF + NROW] = np.broadcast_to(
        np.asarray(b2, dtype=np.float32).T.reshape(OUT, NLVL, 1), (OUT, NLVL, BS)
    ).reshape(OUT, NROW)

    pk = _PK_CONST.copy()
    pk[:, 0:3] = np.tile(np.asarray(p).astype(np.int32), (NLVL, 1))

    in_maps = []
    for c in range(N_CORES):
        xall = np.empty((TOTAL_ROWS, CS), dtype=np.float32)
        for l, x in enumerate(xs):
            shard = np.asarray(x[:, c * CS : (c + 1) * CS], dtype=np.float32)
            xall[BASE[l] : BASE[l + 1]] = shard.transpose(0, 2, 3, 4, 1).reshape(
                BASE[l + 1] - BASE[l], CS
            )
        in_maps.append({"xall": xall, "pk": pk, "wpack": wpack})
    return in_maps


_NC_CACHE = None


def kernel(x0, x1, x2, x3, x4, p, w1, b1, gamma, beta, w2, b2):
    global _NC_CACHE
    if _NC_CACHE is None:
        _NC_CACHE = build_program()
    nc = _NC_CACHE
    in_maps = shard_inputs(x0, x1, x2, x3, x4, p, w1, gamma, beta, w2, b2)
    res = run_bass_kernel_spmd(nc, in_maps, list(range(N_CORES)))
    return np.asarray(res.results[0]["out"], dtype=np.float32)
